# revision 3
# baseline (speedup 1.0000x reference)
"""Trainium2 Bass kernel for AttnAugmentation2d (8 cores, batch-parallel).

Contract: kernel(**inputs) takes FULL inputs
  x [8, 768, 32, 32] f32, rel_w [63, 32] f32, rel_h [63, 32] f32
and returns the FULL output [8, 256, 32, 32] f32.

Sharding: data-parallel over batch — core b computes batch element b.

Per-core computation (channels-on-partitions layout, l = x*32 + y):
  S^T[m, l] = sum_d k[d,m] q[d,l] + Wc[y'(m), l] + Hc[x'(m), l]
  computed as ONE matmul with K=96: k rows plus 0/1 selector rows that
  broadcast the compact relative-position tables Wc/Hc [32, 1024].
  Wc/Hc come from block-diagonal matmuls (4 heads at once, K=128)
  against host-built kron(I4, rel_w.T) shifted tables, per y (resp. x).
  P^T = exp(S^T) (no max-subtraction; logits are O(6), fp32-exp-safe).
  out^T[d, l] = sum_m vT[m, d] P^T[m, l]; an appended ones column in vT
  yields the softmax denominator; 1/denom is broadcast over partitions
  with a K=1 PE matmul and multiplied in on the vector engine.
Matmul operands are bf16 (1 PE cycle/row); accumulation is fp32 PSUM.
"""

import numpy as np
import ml_dtypes

import concourse.bacc as bacc
import concourse.mybir as mybir
from concourse import tile
from concourse.bass_utils import run_bass_kernel_spmd

F32 = mybir.dt.float32
BF16 = mybir.dt.bfloat16
AF = mybir.ActivationFunctionType
BF = np.dtype(ml_dtypes.bfloat16)

NH = 8
HW = 1024
SCALE = 32.0 ** -0.5

_CACHE = {}


def _host_prep_consts(rel_w: np.ndarray, rel_h: np.ndarray):
    # 32 shifted block-diagonal weight tables, one contiguous [128,128]
    # slab per y (walrus requires 1 free dim on matmul weight APs)
    i4 = np.eye(4, dtype=np.float32)
    rwT = np.asarray(rel_w, dtype=np.float32).T
    rhT = np.asarray(rel_h, dtype=np.float32).T
    relw4 = np.concatenate(
        [np.kron(i4, rwT[:, 31 - y : 63 - y]) for y in range(32)], axis=1)
    relh4 = np.concatenate(
        [np.kron(i4, rhT[:, 31 - x : 63 - x]) for x in range(32)], axis=1)
    m = np.arange(HW)
    wsel = (m[None, :] % 32 == np.arange(32)[:, None]).astype(np.float32)
    hsel = (m[None, :] // 32 == np.arange(32)[:, None]).astype(np.float32)
    sel = np.concatenate([wsel, hsel], axis=0)  # [64, 1024]
    consts = dict(
        relw4=relw4.astype(BF),
        relh4=relh4.astype(BF),
        ident=np.eye(32, dtype=np.float32),
        ones1=np.ones((1, 32), dtype=BF),
    )
    return consts, sel.astype(BF)


def _host_prep_core(x_b: np.ndarray, sel_bf: np.ndarray):
    xf = np.ascontiguousarray(np.asarray(x_b, np.float32).reshape(768, HW))
    q4 = (xf[0:256] * SCALE).astype(BF)  # scaled in fp32, rounded once
    lhs = np.empty((NH, 96, HW), dtype=BF)
    for h in range(NH):
        lhs[h, 0:32] = xf[256 + 32 * h : 288 + 32 * h].astype(BF)
        lhs[h, 32:96] = sel_bf
    return dict(q4=q4, lhs=lhs, v=xf[512:768])


def build_nc(niters: int = 1, num_devices: int = 8):
    nc = bacc.Bacc(None, target_bir_lowering=False, debug=False,
                   num_devices=num_devices)

    q4_d = nc.dram_tensor("q4", [256, HW], BF16, kind="ExternalInput").ap()
    lhs_d = nc.dram_tensor("lhs", [NH, 96, HW], BF16, kind="ExternalInput").ap()
    v_d = nc.dram_tensor("v", [256, HW], F32, kind="ExternalInput").ap()
    relw4_d = nc.dram_tensor("relw4", [128, 4096], BF16, kind="ExternalInput").ap()
    relh4_d = nc.dram_tensor("relh4", [128, 4096], BF16, kind="ExternalInput").ap()
    ident_d = nc.dram_tensor("ident", [32, 32], F32, kind="ExternalInput").ap()
    ones1_d = nc.dram_tensor("ones1", [1, 32], BF16, kind="ExternalInput").ap()
    out_d = nc.dram_tensor("out", [256, HW], F32, kind="ExternalOutput").ap()

    with tile.TileContext(nc) as tc:
        with (
            tc.tile_pool(name="consts", bufs=1) as consts,
            tc.tile_pool(name="qpool", bufs=2) as qpool,
            tc.tile_pool(name="lhsp", bufs=3) as lhsp,
            tc.tile_pool(name="rhsp", bufs=3) as rhsp,
            tc.tile_pool(name="vnatp", bufs=2) as vnatp,
            tc.tile_pool(name="vtp", bufs=9) as vtp,
            tc.tile_pool(name="etp", bufs=2) as etp,
            tc.tile_pool(name="fop", bufs=2) as fop,
            tc.tile_pool(name="recp", bufs=2) as recp,
            tc.tile_pool(name="wcs", bufs=2) as wcs,
            tc.tile_pool(name="relps", bufs=1, space="PSUM") as relps,
            tc.tile_pool(name="spp", bufs=1, space="PSUM") as spp,
            tc.tile_pool(name="opp", bufs=1, space="PSUM") as opp,
        ):
            relw4 = consts.tile([128, 4096], BF16, tag="relw4")
            relh4 = consts.tile([128, 4096], BF16, tag="relh4")
            ident = consts.tile([32, 32], F32, tag="ident")
            ones1 = consts.tile([1, 32], BF16, tag="ones1")
            nc.sync.dma_start(relw4[:], relw4_d[:])
            nc.sync.dma_start(relh4[:], relh4_d[:])
            nc.sync.dma_start(ident[:], ident_d[:])
            nc.sync.dma_start(ones1[:], ones1_d[:])

            for it in range(niters):
                qs = [qpool.tile([128, HW], BF16, tag="qs", name=f"qs{it}_{i}")
                      for i in range(2)]
                for g in range(2):
                    nc.sync.dma_start(qs[g][:], q4_d[128 * g : 128 * (g + 1), :])

                # compact rel tables Wc/Hc for both 4-head groups
                # Wc stored y-major (col = 32y + x), Hc natural l-major
                wc_sb, hc_sb = [], []
                for g in range(2):
                    qv = qs[g][:].rearrange("p (x y) -> p x y", y=32)
                    wcp = relps.tile([128, HW], F32, tag="wc", name=f"wcp{it}_{g}")
                    hcp = relps.tile([128, HW], F32, tag="hc", name=f"hcp{it}_{g}")
                    for y in range(32):
                        nc.tensor.matmul(
                            out=wcp[:, 32 * y : 32 * y + 32],
                            lhsT=relw4[:, 128 * y : 128 * y + 128],
                            rhs=qv[:, :, y],
                            start=True, stop=True,
                        )
                    for x in range(32):
                        nc.tensor.matmul(
                            out=hcp[:, 32 * x : 32 * x + 32],
                            lhsT=relh4[:, 128 * x : 128 * x + 128],
                            rhs=qs[g][:, 32 * x : 32 * x + 32],
                            start=True, stop=True,
                        )
                    wt = wcs.tile([128, HW], BF16, tag="wcsb", name=f"wt{it}_{g}")
                    ht = wcs.tile([128, HW], BF16, tag="hcsb", name=f"ht{it}_{g}")
                    nc.vector.tensor_copy(
                        wt[:].rearrange("p (x y) -> p y x", y=32),
                        wcp[:].rearrange("p (y x) -> p y x", x=32),
                    )
                    nc.vector.tensor_copy(ht[:], hcp[:])
                    wc_sb.append(wt)
                    hc_sb.append(ht)

                for h in range(NH):
                    g, r = h // 4, (h % 4) * 32
                    lhs_t = lhsp.tile([96, HW], BF16, tag="lhs", name=f"lhs{it}_{h}")
                    nc.sync.dma_start(lhs_t[:], lhs_d[h])
                    rhs_t = rhsp.tile([96, HW], BF16, tag="rhs", name=f"rhs{it}_{h}")
                    nc.vector.tensor_copy(rhs_t[0:32, :], qs[g][r : r + 32, :])
                    nc.vector.tensor_copy(rhs_t[32:64, :], wc_sb[g][r : r + 32, :])
                    nc.vector.tensor_copy(rhs_t[64:96, :], hc_sb[g][r : r + 32, :])

                    vnat = vnatp.tile([32, HW], F32, tag="vnat", name=f"vn{it}_{h}")
                    nc.sync.dma_start(vnat[:], v_d[32 * h : 32 * h + 32, :])
                    vts = []
                    for m in range(8):
                        vtps_t = spp.tile([128, 32], F32, tag="sp",
                                          name=f"vtps{it}_{h}_{m}")
                        nc.tensor.transpose(
                            vtps_t[:], vnat[:, 128 * m : 128 * (m + 1)], ident[:]
                        )
                        vt = vtp.tile([128, 33], BF16, tag="vt",
                                      name=f"vt{it}_{h}_{m}")
                        nc.vector.tensor_copy(vt[:, 0:32], vtps_t[:])
                        nc.gpsimd.memset(vt[:, 32:33], 1.0)
                        vts.append(vt)

                    op = opp.tile([33, HW], F32, tag="op", name=f"op{it}_{h}")
                    for m in range(8):
                        sp = spp.tile([128, HW], F32, tag="sp",
                                      name=f"sp{it}_{h}_{m}")
                        et = etp.tile([128, HW], BF16, tag="et",
                                      name=f"et{it}_{h}_{m}")
                        for j in range(2):
                            nc.tensor.matmul(
                                out=sp[:, 512 * j : 512 * (j + 1)],
                                lhsT=lhs_t[:, 128 * m : 128 * (m + 1)],
                                rhs=rhs_t[:, 512 * j : 512 * (j + 1)],
                                start=True, stop=True,
                            )
                        for j in range(2):
                            nc.scalar.activation(
                                et[:, 512 * j : 512 * (j + 1)],
                                sp[:, 512 * j : 512 * (j + 1)],
                                AF.Exp,
                            )
                        for j in range(2):
                            nc.tensor.matmul(
                                out=op[:, 512 * j : 512 * (j + 1)],
                                lhsT=vts[m][:],
                                rhs=et[:, 512 * j : 512 * (j + 1)],
                                start=(m == 0), stop=(m == 7),
                            )

                    # normalize: rec = 1/denom (f32), cast bf16, PE-broadcast
                    rec = recp.tile([1, HW], F32, tag="rec", name=f"rec{it}_{h}")
                    nc.vector.reciprocal(rec[:], op[32:33, :])
                    rec16 = recp.tile([1, HW], BF16, tag="rec16",
                                      name=f"rec16{it}_{h}")
                    nc.scalar.copy(rec16[:], rec[:])
                    bc = spp.tile([32, HW], F32, tag="sp", name=f"bc{it}_{h}")
                    for j in range(2):
                        nc.tensor.matmul(
                            out=bc[:, 512 * j : 512 * (j + 1)],
                            lhsT=ones1[:],
                            rhs=rec16[:, 512 * j : 512 * (j + 1)],
                            start=True, stop=True,
                        )
                    bcs = recp.tile([32, HW], F32, tag="bcs", name=f"bcs{it}_{h}")
                    nc.scalar.copy(bcs[:], bc[:])
                    fo = fop.tile([32, HW], F32, tag="fo", name=f"fo{it}_{h}")
                    nc.vector.tensor_mul(fo[:], op[0:32, :], bcs[:])
                    nc.sync.dma_start(out_d[32 * h : 32 * h + 32, :], fo[:])

    nc.compile()
    return nc


def kernel(x: np.ndarray, rel_w: np.ndarray, rel_h: np.ndarray) -> np.ndarray:
    x = np.asarray(x, dtype=np.float32)
    B = x.shape[0]
    n_cores = 8
    assert B == n_cores and x.shape[1:] == (768, 32, 32)

    consts, sel_bf = _host_prep_consts(np.asarray(rel_w), np.asarray(rel_h))
    in_maps = []
    for b in range(n_cores):
        m = dict(consts)
        m.update(_host_prep_core(x[b], sel_bf))
        in_maps.append(m)

    if "nc" not in _CACHE:
        _CACHE["nc"] = build_nc(niters=1, num_devices=n_cores)
    nc = _CACHE["nc"]

    res = run_bass_kernel_spmd(nc, in_maps, list(range(n_cores)))
    out = np.stack([np.asarray(res.results[b]["out"]).reshape(256, 32, 32)
                    for b in range(n_cores)])
    return out.astype(np.float32)


# revision 11
# speedup vs baseline: 1768.1304x; 1768.1304x over previous
"""Trainium2 Bass kernel for AttnAugmentation2d (8 cores, batch-parallel).

Contract: kernel(**inputs) takes FULL inputs
  x [8, 768, 32, 32] f32, rel_w [63, 32] f32, rel_h [63, 32] f32
and returns the FULL output [8, 256, 32, 32] f32.

Sharding: data-parallel over batch — core b computes batch element b.

Per-core computation (channels-on-partitions layout, l = x*32 + y):
  S^T[m, l] = sum_d k[d,m] q[d,l] + Wc[y'(m), l] + Hc[x'(m), l]
  computed as ONE matmul with K=96: k rows plus 0/1 selector rows that
  broadcast the compact relative-position tables Wc/Hc [32, 1024].
  Wc/Hc come from block-diagonal matmuls (4 heads at once, K=128)
  against host-built kron(I4, rel_w.T) shifted tables, per y (resp. x).
  P^T = exp(S^T) (no max-subtraction; logits are O(6), fp32-exp-safe).
  out^T[d, l] = sum_m vT[m, d] P^T[m, l]; an appended ones column in vT
  yields the softmax denominator; 1/denom is broadcast over partitions
  with a K=1 PE matmul and multiplied in on the vector engine.
Matmul operands are bf16 (1 PE cycle/row); accumulation is fp32 PSUM.
"""

import numpy as np
import ml_dtypes

import concourse.bacc as bacc
import concourse.mybir as mybir
from concourse import tile
from concourse.bass_utils import run_bass_kernel_spmd

F32 = mybir.dt.float32
BF16 = mybir.dt.bfloat16
AF = mybir.ActivationFunctionType
BF = np.dtype(ml_dtypes.bfloat16)

NH = 8
HW = 1024
SCALE = 32.0 ** -0.5

_CACHE = {}


def _host_prep_consts(rel_w: np.ndarray, rel_h: np.ndarray):
    # 32 shifted block-diagonal weight tables, one contiguous [128,128]
    # slab per y (walrus requires 1 free dim on matmul weight APs)
    i4 = np.eye(4, dtype=np.float32)
    rwT = np.asarray(rel_w, dtype=np.float32).T
    rhT = np.asarray(rel_h, dtype=np.float32).T
    relw4 = np.kron(i4, rwT)  # [128, 252]
    relh4 = np.kron(i4, rhT)
    m = np.arange(HW)
    wsel = (m[None, :] % 32 == np.arange(32)[:, None]).astype(np.float32)
    hsel = (m[None, :] // 32 == np.arange(32)[:, None]).astype(np.float32)
    sel = np.concatenate([wsel, hsel], axis=0)  # [64, 1024]
    consts = dict(
        relw4=relw4.astype(BF),
        relh4=relh4.astype(BF),
        ones1=np.ones((1, 32), dtype=BF),
    )
    return consts, sel.astype(BF)


def _host_prep_core(x_b: np.ndarray, sel_bf: np.ndarray):
    xf = np.ascontiguousarray(np.asarray(x_b, np.float32).reshape(768, HW))
    q4 = (xf[0:256] * SCALE).astype(BF)  # scaled in fp32, rounded once
    lhs = np.empty((NH, 96, HW), dtype=BF)
    for h in range(NH):
        lhs[h, 0:32] = xf[256 + 32 * h : 288 + 32 * h].astype(BF)
        lhs[h, 32:96] = sel_bf
    v = np.zeros((NH, 48, HW), dtype=BF)
    for h in range(NH):
        v[h, 0:32] = xf[512 + 32 * h : 544 + 32 * h].astype(BF)
        v[h, 32] = 1.0
    return dict(q4=q4, lhs=lhs, v=v)


def build_nc(niters: int = 1, num_devices: int = 8):
    nc = bacc.Bacc(None, target_bir_lowering=False, debug=False,
                   num_devices=num_devices)

    q4_d = nc.dram_tensor("q4", [256, HW], BF16, kind="ExternalInput").ap()
    lhs_d = nc.dram_tensor("lhs", [NH, 96, HW], BF16, kind="ExternalInput").ap()
    v_d = nc.dram_tensor("v", [NH, 48, HW], BF16, kind="ExternalInput").ap()
    relw4_d = nc.dram_tensor("relw4", [128, 252], BF16, kind="ExternalInput").ap()
    relh4_d = nc.dram_tensor("relh4", [128, 252], BF16, kind="ExternalInput").ap()
    ones1_d = nc.dram_tensor("ones1", [1, 32], BF16, kind="ExternalInput").ap()
    out_d = nc.dram_tensor("out", [256, HW], F32, kind="ExternalOutput").ap()

    with tile.TileContext(nc) as tc:
        with (
            tc.tile_pool(name="consts", bufs=1) as consts,
            tc.tile_pool(name="qpool", bufs=2) as qpool,
            tc.tile_pool(name="lhsp", bufs=4) as lhsp,
            tc.tile_pool(name="rhsp", bufs=4) as rhsp,
            tc.tile_pool(name="vtallp", bufs=2) as vtallp,
            tc.tile_pool(name="etp", bufs=3) as etp,
            tc.tile_pool(name="fop", bufs=2) as fop,
            tc.tile_pool(name="recp", bufs=2) as recp,
            tc.tile_pool(name="wcs", bufs=2) as wcs,
            tc.tile_pool(name="spp", bufs=2, space="PSUM") as spp,
            tc.tile_pool(name="opp", bufs=2, space="PSUM") as opp,
        ):
            relw4c = consts.tile([128, 252], BF16, tag="relw4c")
            relh4c = consts.tile([128, 252], BF16, tag="relh4c")
            ones1 = consts.tile([1, 32], BF16, tag="ones1")
            nc.sync.dma_start(relw4c[:], relw4_d[:])
            nc.sync.dma_start(relh4c[:], relh4_d[:])
            nc.sync.dma_start(ones1[:], ones1_d[:])
            # expand the 32 shifted block-diagonal slabs on-device (DVE,
            # off the ACT critical path; walrus needs 1-free-dim weights)
            relw4 = consts.tile([128, 4096], BF16, tag="relw4")
            relh4 = consts.tile([128, 4096], BF16, tag="relh4")
            relw4c_v = relw4c[:].rearrange("p (h j) -> p h j", h=4)
            relh4c_v = relh4c[:].rearrange("p (h j) -> p h j", h=4)
            for y in range(32):
                nc.vector.tensor_copy(
                    relw4[:, 128 * y : 128 * y + 128]
                        .rearrange("p (h j) -> p h j", h=4),
                    relw4c_v[:, :, 31 - y : 63 - y],
                )
                nc.vector.tensor_copy(
                    relh4[:, 128 * y : 128 * y + 128]
                        .rearrange("p (h j) -> p h j", h=4),
                    relh4c_v[:, :, 31 - y : 63 - y],
                )

            for it in range(niters):
                qs = [qpool.tile([128, HW], BF16, tag="qs", name=f"qs{it}_{i}")
                      for i in range(2)]
                for g in range(2):
                    for half in range(2):
                        nc.sync.dma_start(
                            qs[g][:, 512 * half : 512 * (half + 1)],
                            q4_d[128 * g : 128 * (g + 1),
                                 512 * half : 512 * (half + 1)])

                # compact rel tables Wc/Hc for both 4-head groups
                # Wc stored y-major (col = 32y + x), Hc natural l-major
                wc_sb, hc_sb = [], []
                for g in range(2):
                    qv = qs[g][:].rearrange("p (x y) -> p x y", y=32)
                    wcp = spp.tile([128, HW], F32, tag="sp", name=f"wcp{it}_{g}")
                    hcp = spp.tile([128, HW], F32, tag="sp", name=f"hcp{it}_{g}")
                    for x in range(32):
                        nc.tensor.matmul(
                            out=hcp[:, 32 * x : 32 * x + 32],
                            lhsT=relh4[:, 128 * x : 128 * x + 128],
                            rhs=qs[g][:, 32 * x : 32 * x + 32],
                            start=True, stop=True,
                        )
                    for y in range(32):
                        nc.tensor.matmul(
                            out=wcp[:, 32 * y : 32 * y + 32],
                            lhsT=relw4[:, 128 * y : 128 * y + 128],
                            rhs=qv[:, :, y],
                            start=True, stop=True,
                        )
                    wt = wcs.tile([128, HW], BF16, tag="wcsb", name=f"wt{it}_{g}")
                    ht = wcs.tile([128, HW], BF16, tag="hcsb", name=f"ht{it}_{g}")
                    nc.vector.tensor_copy(
                        wt[:].rearrange("p (x y) -> p y x", y=32),
                        wcp[:].rearrange("p (y x) -> p y x", x=32),
                    )
                    nc.vector.tensor_copy(ht[:], hcp[:])
                    wc_sb.append(wt)
                    hc_sb.append(ht)

                for h in range(NH):
                    g, r = h // 4, (h % 4) * 32
                    lhs_t = lhsp.tile([96, HW], BF16, tag="lhs", name=f"lhs{it}_{h}")
                    nc.sync.dma_start(lhs_t[:], lhs_d[h])
                    rhs_t = rhsp.tile([96, HW], BF16, tag="rhs", name=f"rhs{it}_{h}")
                    nc.sync.dma_start(rhs_t[0:32, :], q4_d[128 * g + r : 128 * g + r + 32, :])
                    nc.vector.tensor_copy(rhs_t[32:64, :], wc_sb[g][r : r + 32, :])
                    nc.vector.tensor_copy(rhs_t[64:96, :], hc_sb[g][r : r + 32, :])

                    vt_all = vtallp.tile([128, 384], BF16, tag="vtall",
                                         name=f"vta{it}_{h}")
                    vtv = vt_all[:].rearrange("p (m j) -> p m j", j=48)
                    nc.sync.dma_start_transpose(vtv, v_d[h])

                    op = opp.tile([33, HW], F32, tag="op", name=f"op{it}_{h}")
                    for m in range(8):
                        sp = spp.tile([128, HW], F32, tag="sp",
                                      name=f"sp{it}_{h}_{m}")
                        et = etp.tile([128, HW], BF16, tag="et",
                                      name=f"et{it}_{h}_{m}")
                        for j in range(2):
                            nc.tensor.matmul(
                                out=sp[:, 512 * j : 512 * (j + 1)],
                                lhsT=lhs_t[:, 128 * m : 128 * (m + 1)],
                                rhs=rhs_t[:, 512 * j : 512 * (j + 1)],
                                start=True, stop=True,
                            )
                        nc.scalar.activation(et[:], sp[:], AF.Exp)
                        for j in range(2):
                            nc.tensor.matmul(
                                out=op[:, 512 * j : 512 * (j + 1)],
                                lhsT=vtv[:, m, 0:33],
                                rhs=et[:, 512 * j : 512 * (j + 1)],
                                start=(m == 0), stop=(m == 7),
                            )

                    # normalize: gpsimd-broadcast denom, reciprocal, multiply
                    den = recp.tile([1, HW], F32, tag="den",
                                    name=f"den{it}_{h}")
                    nc.vector.tensor_copy(den[:], op[32:33, :])
                    denb = recp.tile([32, HW], F32, tag="denb",
                                     name=f"denb{it}_{h}")
                    nc.gpsimd.partition_broadcast(denb[:], den[:])
                    recb = recp.tile([32, HW], F32, tag="recb",
                                     name=f"recb{it}_{h}")
                    nc.vector.reciprocal(recb[:], denb[:])
                    fo = fop.tile([32, HW], F32, tag="fo", name=f"fo{it}_{h}")
                    nc.vector.tensor_mul(fo[:], op[0:32, :], recb[:])
                    nc.sync.dma_start(out_d[32 * h : 32 * h + 32, :], fo[:])

    nc.compile()
    return nc


def kernel(x: np.ndarray, rel_w: np.ndarray, rel_h: np.ndarray) -> np.ndarray:
    x = np.asarray(x, dtype=np.float32)
    B = x.shape[0]
    n_cores = 8
    assert B == n_cores and x.shape[1:] == (768, 32, 32)

    consts, sel_bf = _host_prep_consts(np.asarray(rel_w), np.asarray(rel_h))
    in_maps = []
    for b in range(n_cores):
        m = dict(consts)
        m.update(_host_prep_core(x[b], sel_bf))
        in_maps.append(m)

    if "nc" not in _CACHE:
        _CACHE["nc"] = build_nc(niters=1, num_devices=n_cores)
    nc = _CACHE["nc"]

    res = run_bass_kernel_spmd(nc, in_maps, list(range(n_cores)))
    out = np.stack([np.asarray(res.results[b]["out"]).reshape(256, 32, 32)
                    for b in range(n_cores)])
    return out.astype(np.float32)


# revision 13
# speedup vs baseline: 9396.4185x; 5.3143x over previous
"""Trainium2 Bass kernel for AttnAugmentation2d (8 cores, batch-parallel).

Contract: kernel(**inputs) takes FULL inputs
  x [8, 768, 32, 32] f32, rel_w [63, 32] f32, rel_h [63, 32] f32
and returns the FULL output [8, 256, 32, 32] f32.

Sharding: data-parallel over batch — core b computes batch element b.

Per-core computation (channels-on-partitions layout, l = x*32 + y):
  S^T[m, l] = sum_d k[d,m] q[d,l] + Wc[y'(m), l] + Hc[x'(m), l]
  computed as ONE matmul with K=96: k rows plus 0/1 selector rows that
  broadcast the compact relative-position tables Wc/Hc [32, 1024].
  Wc/Hc come from block-diagonal matmuls (4 heads at once, K=128)
  against host-built kron(I4, rel_w.T) shifted tables, per y (resp. x).
  P^T = exp(S^T) (no max-subtraction; logits are O(6), fp32-exp-safe).
  out^T[d, l] = sum_m vT[m, d] P^T[m, l]; an appended ones column in vT
  (host-baked, DMA-transposed HBM->SBUF) yields the softmax denominator;
  the denominator is broadcast over partitions on GPSIMD, reciprocated
  and multiplied in on the vector engine.
Matmul operands are bf16 (1 PE cycle/row); accumulation is fp32 PSUM.
"""

import numpy as np
import ml_dtypes

import concourse.bacc as bacc
import concourse.mybir as mybir
from concourse import tile
from concourse.bass_utils import run_bass_kernel_spmd

F32 = mybir.dt.float32
BF16 = mybir.dt.bfloat16
AF = mybir.ActivationFunctionType
BF = np.dtype(ml_dtypes.bfloat16)

NH = 8
HW = 1024
SCALE = 32.0 ** -0.5

_CACHE = {}


def _host_prep_consts(rel_w: np.ndarray, rel_h: np.ndarray):
    # 32 shifted block-diagonal weight tables, one contiguous [128,128]
    # slab per y (walrus requires 1 free dim on matmul weight APs)
    i4 = np.eye(4, dtype=np.float32)
    rwT = np.asarray(rel_w, dtype=np.float32).T
    rhT = np.asarray(rel_h, dtype=np.float32).T
    relw4 = np.kron(i4, rwT)  # [128, 252]
    relh4 = np.kron(i4, rhT)
    m = np.arange(HW)
    wsel = (m[None, :] % 32 == np.arange(32)[:, None]).astype(np.float32)
    hsel = (m[None, :] // 32 == np.arange(32)[:, None]).astype(np.float32)
    sel = np.concatenate([wsel, hsel], axis=0)  # [64, 1024]
    consts = dict(
        relw4=relw4.astype(BF),
        relh4=relh4.astype(BF),
    )
    return consts, sel.astype(BF)


def _host_prep_core(x_b: np.ndarray, sel_bf: np.ndarray):
    xf = np.ascontiguousarray(np.asarray(x_b, np.float32).reshape(768, HW))
    q4 = (xf[0:256] * SCALE).astype(BF)  # scaled in fp32, rounded once
    lhs = np.empty((NH, 96, HW), dtype=BF)
    for h in range(NH):
        lhs[h, 0:32] = xf[256 + 32 * h : 288 + 32 * h].astype(BF)
        lhs[h, 32:96] = sel_bf
    v = np.zeros((NH, 48, HW), dtype=BF)
    for h in range(NH):
        v[h, 0:32] = xf[512 + 32 * h : 544 + 32 * h].astype(BF)
        v[h, 32] = 1.0
    return dict(q4=q4, lhs=lhs, v=v)


def build_nc(niters: int = 1, num_devices: int = 8):
    nc = bacc.Bacc(None, target_bir_lowering=False, debug=False,
                   num_devices=num_devices)

    q4_d = nc.dram_tensor("q4", [256, HW], BF16, kind="ExternalInput").ap()
    lhs_d = nc.dram_tensor("lhs", [NH, 96, HW], BF16, kind="ExternalInput").ap()
    v_d = nc.dram_tensor("v", [NH, 48, HW], BF16, kind="ExternalInput").ap()
    relw4_d = nc.dram_tensor("relw4", [128, 252], BF16, kind="ExternalInput").ap()
    relh4_d = nc.dram_tensor("relh4", [128, 252], BF16, kind="ExternalInput").ap()
    out_d = nc.dram_tensor("out", [256, HW], F32, kind="ExternalOutput").ap()

    with tile.TileContext(nc) as tc:
        with (
            tc.tile_pool(name="consts", bufs=1) as consts,
            tc.tile_pool(name="qpool", bufs=2) as qpool,
            tc.tile_pool(name="lhsp", bufs=4) as lhsp,
            tc.tile_pool(name="rhsp", bufs=4) as rhsp,
            tc.tile_pool(name="vtallp", bufs=2) as vtallp,
            tc.tile_pool(name="etp", bufs=3) as etp,
            tc.tile_pool(name="fop", bufs=2) as fop,
            tc.tile_pool(name="recp", bufs=2) as recp,
            tc.tile_pool(name="wcs", bufs=2) as wcs,
            tc.tile_pool(name="spp", bufs=2, space="PSUM") as spp,
            tc.tile_pool(name="opp", bufs=2, space="PSUM") as opp,
        ):
            relh4c = consts.tile([128, 252], BF16, tag="relh4c")
            relw4c = consts.tile([128, 252], BF16, tag="relw4c")
            nc.sync.dma_start(relh4c[:], relh4_d[:])
            nc.sync.dma_start(relw4c[:], relw4_d[:])
            # expand the 32 shifted block-diagonal slabs on-device (DVE,
            # off the ACT critical path; walrus needs 1-free-dim weights)
            relw4 = consts.tile([128, 4096], BF16, tag="relw4")
            relh4 = consts.tile([128, 4096], BF16, tag="relh4")
            relw4c_v = relw4c[:].rearrange("p (h j) -> p h j", h=4)
            relh4c_v = relh4c[:].rearrange("p (h j) -> p h j", h=4)
            for y in range(32):
                nc.vector.tensor_copy(
                    relh4[:, 128 * y : 128 * y + 128]
                        .rearrange("p (h j) -> p h j", h=4),
                    relh4c_v[:, :, 31 - y : 63 - y],
                )
            for y in range(32):
                nc.vector.tensor_copy(
                    relw4[:, 128 * y : 128 * y + 128]
                        .rearrange("p (h j) -> p h j", h=4),
                    relw4c_v[:, :, 31 - y : 63 - y],
                )

            for it in range(niters):
                qs = [qpool.tile([128, HW], BF16, tag="qs", name=f"qs{it}_{i}")
                      for i in range(2)]
                for g in range(2):
                    for half in range(2):
                        nc.sync.dma_start(
                            qs[g][:, 512 * half : 512 * (half + 1)],
                            q4_d[128 * g : 128 * (g + 1),
                                 512 * half : 512 * (half + 1)])

                # compact rel tables Wc/Hc for both 4-head groups
                # Wc stored y-major (col = 32y + x), Hc natural l-major
                wc_sb, hc_sb = [], []
                for g in range(2):
                    qv = qs[g][:].rearrange("p (x y) -> p x y", y=32)
                    wcp = spp.tile([128, HW], F32, tag="sp", name=f"wcp{it}_{g}")
                    hcp = spp.tile([128, HW], F32, tag="sp", name=f"hcp{it}_{g}")
                    for x in range(32):
                        nc.tensor.matmul(
                            out=hcp[:, 32 * x : 32 * x + 32],
                            lhsT=relh4[:, 128 * x : 128 * x + 128],
                            rhs=qs[g][:, 32 * x : 32 * x + 32],
                            start=True, stop=True,
                        )
                    for y in range(32):
                        nc.tensor.matmul(
                            out=wcp[:, 32 * y : 32 * y + 32],
                            lhsT=relw4[:, 128 * y : 128 * y + 128],
                            rhs=qv[:, :, y],
                            start=True, stop=True,
                        )
                    wt = wcs.tile([128, HW], BF16, tag="wcsb", name=f"wt{it}_{g}")
                    ht = wcs.tile([128, HW], BF16, tag="hcsb", name=f"ht{it}_{g}")
                    nc.vector.tensor_copy(
                        wt[:].rearrange("p (x y) -> p y x", y=32),
                        wcp[:].rearrange("p (y x) -> p y x", x=32),
                    )
                    nc.vector.tensor_copy(ht[:], hcp[:])
                    wc_sb.append(wt)
                    hc_sb.append(ht)

                for h in range(NH):
                    g, r = h // 4, (h % 4) * 32
                    lhs_t = lhsp.tile([96, HW], BF16, tag="lhs", name=f"lhs{it}_{h}")
                    nc.gpsimd.dma_start(lhs_t[:], lhs_d[h])
                    rhs_t = rhsp.tile([96, HW], BF16, tag="rhs", name=f"rhs{it}_{h}")
                    nc.sync.dma_start(rhs_t[0:32, :], q4_d[128 * g + r : 128 * g + r + 32, :])
                    nc.vector.tensor_copy(rhs_t[32:64, :], wc_sb[g][r : r + 32, :])
                    nc.vector.tensor_copy(rhs_t[64:96, :], hc_sb[g][r : r + 32, :])

                    vt_all = vtallp.tile([128, 384], BF16, tag="vtall",
                                         name=f"vta{it}_{h}")
                    vtv = vt_all[:].rearrange("p (m j) -> p m j", j=48)
                    nc.sync.dma_start_transpose(vtv, v_d[h])

                    op = opp.tile([33, HW], F32, tag="op", name=f"op{it}_{h}")
                    for m in range(8):
                        sp = spp.tile([128, HW], F32, tag="sp",
                                      name=f"sp{it}_{h}_{m}")
                        et = etp.tile([128, HW], BF16, tag="et",
                                      name=f"et{it}_{h}_{m}")
                        for j in range(2):
                            nc.tensor.matmul(
                                out=sp[:, 512 * j : 512 * (j + 1)],
                                lhsT=lhs_t[:, 128 * m : 128 * (m + 1)],
                                rhs=rhs_t[:, 512 * j : 512 * (j + 1)],
                                start=True, stop=True,
                            )
                        nc.scalar.activation(et[:], sp[:], AF.Exp)
                        for j in range(2):
                            nc.tensor.matmul(
                                out=op[:, 512 * j : 512 * (j + 1)],
                                lhsT=vtv[:, m, 0:33],
                                rhs=et[:, 512 * j : 512 * (j + 1)],
                                start=(m == 0), stop=(m == 7),
                            )

                    # normalize: gpsimd-broadcast denom, reciprocal, multiply
                    den = recp.tile([1, HW], F32, tag="den",
                                    name=f"den{it}_{h}")
                    nc.vector.tensor_copy(den[:], op[32:33, :])
                    denb = recp.tile([32, HW], F32, tag="denb",
                                     name=f"denb{it}_{h}")
                    nc.gpsimd.partition_broadcast(denb[:], den[:])
                    recb = recp.tile([32, HW], F32, tag="recb",
                                     name=f"recb{it}_{h}")
                    nc.vector.reciprocal(recb[:], denb[:])
                    fo = fop.tile([32, HW], F32, tag="fo", name=f"fo{it}_{h}")
                    nc.vector.tensor_mul(fo[:], op[0:32, :], recb[:])
                    nc.sync.dma_start(out_d[32 * h : 32 * h + 32, :], fo[:])

    nc.compile()
    return nc


def kernel(x: np.ndarray, rel_w: np.ndarray, rel_h: np.ndarray) -> np.ndarray:
    x = np.asarray(x, dtype=np.float32)
    B = x.shape[0]
    n_cores = 8
    assert B == n_cores and x.shape[1:] == (768, 32, 32)

    consts, sel_bf = _host_prep_consts(np.asarray(rel_w), np.asarray(rel_h))
    in_maps = []
    for b in range(n_cores):
        m = dict(consts)
        m.update(_host_prep_core(x[b], sel_bf))
        in_maps.append(m)

    if "nc" not in _CACHE:
        _CACHE["nc"] = build_nc(niters=1, num_devices=n_cores)
    nc = _CACHE["nc"]

    res = run_bass_kernel_spmd(nc, in_maps, list(range(n_cores)))
    out = np.stack([np.asarray(res.results[b]["out"]).reshape(256, 32, 32)
                    for b in range(n_cores)])
    return out.astype(np.float32)
